# revision 1
# baseline (speedup 1.0000x reference)
"""Distributed TRN2 Bass kernel for nn_Autoencoder_34995393527840 (retrieval_knn).

Core idea: quantized d2_enc values are bit-stuffed into the low 10 mantissa
bits of the d2_ini row values, so row-wise top-64 extraction (DVE
max8/match_replace) yields (d2_ini, d2_enc) pairs directly, eliminating the
take_along_axis gather entirely.
"""

import numpy as np

N, D = 4096, 784
NCORES = 8
RPC = N // NCORES          # 512 rows per core
NT = RPC // 128            # 4 row-tiles per core
KSH = 787                  # shard rows: 784 xT + sqh + sql + ones
C_SHIFT = 512.0
QBITS = 10
QMAX = (1 << QBITS) - 1
MASK_HI = 0xFFFFFFFF ^ QMAX
HALF_BUCKET = (QMAX + 1) / 2 * 2.0 ** -23 * 256

_CACHE = {}
TRACE = False


def _build(dbg=False):
    import concourse.bacc as bacc
    import concourse.mybir as mybir
    from concourse.tile import TileContext

    f32 = mybir.dt.float32
    bf16 = mybir.dt.bfloat16
    u32 = mybir.dt.uint32
    AF = mybir.ActivationFunctionType
    OP = mybir.AluOpType
    AX = mybir.AxisListType

    nc = bacc.Bacc("TRN2", target_bir_lowering=False, debug=False)

    xq_ext = nc.declare_dram_parameter("xq", [RPC, D], f32, isOutput=False)
    w1l_ext = nc.declare_dram_parameter("w1l", [7, 3, 128], bf16, isOutput=False)
    w2l_ext = nc.declare_dram_parameter("w2l", [193, 3, 128], bf16, isOutput=False)
    w3l_ext = nc.declare_dram_parameter("w3l", [193, 3, 64], bf16, isOutput=False)
    dwl_ext = nc.declare_dram_parameter("dwl", [785, 16], bf16, isOutput=False)
    idT_ext = nc.declare_dram_parameter("idT", [128, 128], bf16, isOutput=False)
    out_ext = nc.declare_dram_parameter("out", [1, 8], f32, isOutput=True)
    if dbg:
        dbgE_ext = nc.declare_dram_parameter("dbgE", [17, 512], f32, isOutput=True)
        dbgS_ext = nc.declare_dram_parameter("dbgS", [128, 32], f32, isOutput=True)
        dbgV_ext = nc.declare_dram_parameter("dbgV", [128, 64], f32, isOutput=True)
        dbgH_ext = nc.declare_dram_parameter("dbgH", [64, 7 * 512], f32, isOutput=True)

    with TileContext(nc) as tc:
        with (
            tc.tile_pool(name="sb", bufs=1) as sb,
            tc.tile_pool(name="ps", bufs=1, space="PSUM") as ps,
            tc.tile_pool(name="dr", bufs=1, space="DRAM") as dr,
        ):
            shard_dram = dr.tile([KSH, RPC], bf16)
            agx_dram = dr.tile([NCORES, KSH, RPC], bf16, addr_space="Shared")
            sharde_dram = dr.tile([17, RPC], f32)
            age_dram = dr.tile([NCORES, 17, RPC], f32, addr_space="Shared")
            rs_dram = dr.tile([1, 1], f32)
            sev_dram = dr.tile([128, 1], f32)
            rsum_dram = dr.tile([1, 1], f32, addr_space="Shared")
            h1_dram = dr.tile([32, 14, 14, RPC], bf16)   # [ci, y, x, n]
            h2_dram = dr.tile([32, 7, 7, RPC], bf16)     # [ci, y, x, n]
            zdram = dr.tile([32, 16384], bf16)
            odram = dr.tile([32, 16384], bf16)

            RG = [list(range(NCORES))]

            # ---------- zeros / ones scratch ----------
            zsb = sb.tile([128, 512], bf16)
            nc.vector.memset(zsb, 0.0)
            zdv = zdram.rearrange("p (c f) -> p c f", c=32)
            for c in range(32):
                nc.sync.dma_start(out=zdv[:, c, :], in_=zsb[0:32, :])
            osbt = sb.tile([128, 512], bf16, tag="osb")
            nc.vector.memset(osbt, 1.0)
            odv = odram.rearrange("p (c f) -> p c f", c=32)
            for c in range(32):
                nc.sync.dma_start(out=odv[:, c, :], in_=osbt[0:32, :])
            osb = osbt[0:1, :]

            # ============================================================
            # Stage 0: own x -> bf16, sq, PE-transpose -> shard -> AllGather
            # ============================================================
            idT = sb.tile([128, 128], bf16)
            nc.sync.dma_start(out=idT, in_=idT_ext[:, :])

            XBC = 788
            sq_q = sb.tile([128, NT], f32)
            xbf_t = []
            xbf_tags = ["v0", "v1", "vm", "h1c"]
            for t in range(NT):
                xt = sb.tile([128, D], f32, tag="t1", bufs=2)
                nc.sync.dma_start(out=xt, in_=xq_ext[128 * t:128 * (t + 1), :])
                xb = sb.tile([128, XBC], bf16, tag=xbf_tags[t])
                sqs = sb.tile([128, D], f32, tag="t1", bufs=2)
                nc.scalar.activation(out=xb[:, 0:D], in_=xt, func=AF.Copy)
                nc.scalar.activation(out=sqs, in_=xt, func=AF.Square,
                                     accum_out=sq_q[:, t:t + 1])
                sqh32 = sb.tile([128, 2], f32, tag=f"sqh32{t}")
                nc.vector.tensor_copy(xb[:, D:D + 1], sq_q[:, t:t + 1])
                nc.vector.tensor_copy(sqh32[:, 0:1], xb[:, D:D + 1])
                nc.vector.tensor_sub(sqh32[:, 1:2], sq_q[:, t:t + 1], sqh32[:, 0:1])
                nc.vector.tensor_copy(xb[:, D + 1:D + 2], sqh32[:, 1:2])
                nc.vector.memset(xb[:, D + 2:D + 3], 1.0)
                nc.vector.memset(xb[:, D + 3:XBC], 0.0)
                xbf_t.append(xb)

            shard_sb = []
            for c in range(7):
                c0 = 128 * c
                cw = min(128, XBC - c0)
                rows = min(cw, KSH - c0)
                pt = ps.tile([128, 512], bf16, tag="big", bufs=1)
                for t in range(NT):
                    nc.tensor.transpose(pt[0:cw, 128 * t:128 * (t + 1)],
                                        xbf_t[t][:, c0:c0 + cw], idT)
                sh = sb.tile([128, 512], bf16, tag=f"shard{c}")
                nc.scalar.activation(out=sh[0:rows, :], in_=pt[0:rows, :], func=AF.Copy)
                nc.sync.dma_start(out=shard_dram[c0:c0 + rows, :], in_=sh[0:rows, :])
                shard_sb.append((sh, rows))

            nc.gpsimd.collective_compute(
                "AllGather", OP.bypass, replica_groups=RG,
                ins=[shard_dram[:, :].opt()], outs=[agx_dram[:, :, :].opt()])

            # ============================================================
            # Stage 1: conv1 + maxpool -> h1 [32, 14, 16, 512]
            # M = (yg4, co32); K = (yoff6, kx3) + bias; 4 yb-pairs x 4 n-chunks
            # ============================================================
            w1t = sb.tile([128, 3, 128], bf16, tag="w1")
            w1 = w1t[0:7, :, :]
            nc.sync.dma_start(out=w1, in_=w1l_ext[:, :, :])
            srcx = shard_dram[0:784, :].rearrange("(y x) n -> y x n", y=28)

            # ---- x-Gram lhs tiles + stuff buffers + gram emitter (interleaved) ----
            lhq = []
            for kt in range(7):
                if kt < 6:
                    l = sb.tile([128, 512], bf16, tag=f"lhq{kt}")
                    nc.scalar.activation(out=l, in_=shard_sb[kt][0], func=AF.Copy,
                                         scale=-2.0)
                else:
                    lt6 = sb.tile([128, 512], bf16, tag="lhq6")
                    l = lt6[0:19, :]
                    nc.scalar.activation(out=l[0:16, :], in_=shard_sb[6][0][0:16, :],
                                         func=AF.Copy, scale=-2.0)
                    nc.sync.dma_start(
                        out=l[16:18, :],
                        in_=odram[0:1, 0:1024].rearrange("a (b n) -> a b n", b=2))
                    csb = sb.tile([128, 512], bf16, tag="csb")
                    nc.vector.memset(csb[0:32, :], C_SHIFT)
                    nc.sync.dma_start(out=l[18:19, :], in_=csb[0:1, :])
                lhq.append(l)
            stuff_dram = {}
            for m_ in range(NT):
                stuff_dram[m_] = dr.tile([128, 4096], u32, name=f"stuffd{m_}")

            def emit_gram_ch(ch):
                gms_ = {}
                for m_ in range(NT):
                    gms_[m_] = ps.tile([128, 512], f32, tag="gps", bufs=4,
                                       name=f"gm{m_}{ch}")
                for kt_ in range(7):
                    rows_ = 128 if kt_ < 6 else 19
                    rt = sb.tile([128, 512], bf16, tag="rt", bufs=2)
                    r_ = rt[0:rows_, :]
                    nc.sync.dma_start(
                        out=r_, in_=agx_dram[ch, 128 * kt_:128 * kt_ + rows_, :])
                    for m_ in range(NT):
                        nc.tensor.matmul(
                            gms_[m_], lhq[kt_][:, 128 * m_:128 * (m_ + 1)], r_,
                            start=(kt_ == 0), stop=(kt_ == 6))
                for m_ in range(NT):
                    sev_ = sb.tile([128, 512], u32, tag="stev", bufs=4,
                                   name=f"stev{m_}{ch}")
                    nc.vector.tensor_scalar(
                        out=sev_, in0=gms_[m_].bitcast(u32), scalar1=MASK_HI,
                        scalar2=None, op0=OP.bitwise_and)
                    nc.sync.dma_start(
                        out=stuff_dram[m_][:, 512 * ch:512 * (ch + 1)], in_=sev_)

            # conv1: kx-in-free; patches [7 rows = yoff6+bias, 16x, 512n]
            for ybp in (0, 2, 4, 6):
                ybs = [ybp] if ybp == 6 else [ybp, ybp + 1]
                for xh in range(2):
                    t1s = []
                    for yb in ybs:
                        p1t = sb.tile([128, 16 * 512], bf16, tag="cp", bufs=2,
                                      name=f"p1t{yb}{xh}")
                        p1 = p1t[0:7, :]
                        p1v = p1.rearrange("p (x n) -> p x n", x=16)
                        nc.scalar.dma_start(out=p1[6:7, :],
                                            in_=odram[0:1, 0:16 * 512])
                        yo_lo = 1 if yb == 0 else 0
                        yo_hi = 4 if yb == 6 else 5
                        for yo_z in range(6):
                            if not (yo_lo <= yo_z <= yo_hi):
                                nc.scalar.dma_start(
                                    out=p1v[yo_z:yo_z + 1, :, :],
                                    in_=zdram[0:1, 0:16 * 512].rearrange(
                                        "p (x n) -> p x n", x=16))
                        # x range: cols c=0..15 <-> in_x = 14*xh + c - 1
                        c_lo = 1 if xh == 0 else 0
                        c_hi = 14 if xh == 1 else 15
                        if xh == 0:
                            nc.scalar.dma_start(
                                out=p1v[yo_lo:yo_hi + 1, 0:1, :],
                                in_=zdram[0:yo_hi - yo_lo + 1, 0:512].rearrange(
                                    "p (x n) -> p x n", x=1))
                        else:
                            nc.scalar.dma_start(
                                out=p1v[yo_lo:yo_hi + 1, 15:16, :],
                                in_=zdram[0:yo_hi - yo_lo + 1, 0:512].rearrange(
                                    "p (x n) -> p x n", x=1))
                        nc.sync.dma_start(
                            out=p1v[yo_lo:yo_hi + 1, c_lo:c_hi + 1, :],
                            in_=srcx[4 * yb + yo_lo - 1:4 * yb + yo_hi, :, :]
                                .rearrange("y x n -> y (x n)")
                                [:, (14 * xh + c_lo - 1) * 512:
                                    (14 * xh + c_hi) * 512]
                                .rearrange("y (x n) -> y x n", n=512))
                        t1 = sb.tile([128, 7 * 512], bf16, tag="t1", bufs=2,
                                     name=f"t1_{yb}{xh}")
                        t1v = t1.rearrange("p (x n) -> p x n", x=7)
                        for (g0, g1) in ((0, 4), (4, 8), (8, 12), (12, 14)):
                            nxp = g1 - g0
                            pg = ps.tile([128, 2048], f32, tag="big", bufs=1)
                            for xs in range(g0, g1):
                                for kx in range(3):
                                    nc.tensor.matmul(
                                        pg[:, (xs - g0) * 512:(xs - g0 + 1) * 512],
                                        w1[:, kx, :], p1v[:, xs + kx, :],
                                        start=(kx == 0), stop=(kx == 2))
                            e1 = sb.tile([128, 2048], bf16, tag="e1", bufs=2)
                            nc.scalar.activation(out=e1[:, 0:nxp * 512],
                                                 in_=pg[:, 0:nxp * 512],
                                                 func=AF.Relu)
                            e1v = e1.rearrange("p (x n) -> p x n", x=4)
                            nc.vector.tensor_tensor(
                                out=t1v[:, g0 // 2:g1 // 2, :],
                                in0=e1v[:, 0:nxp:2, :], in1=e1v[:, 1:nxp:2, :],
                                op=OP.max)
                        t1s.append(t1v)
                    v0 = sb.tile([128, 7 * 512], bf16, tag="v0")
                    v1 = sb.tile([128, 7 * 512], bf16, tag="v1")
                    for j, t1v in enumerate(t1s):
                        for i, (ga, gb) in enumerate(((0, 1), (2, 3))):
                            bx = 2 * j + i
                            nc.sync.dma_start(out=v0[32 * bx:32 * bx + 32, :],
                                              in_=t1v[32 * ga:32 * ga + 32, :, :])
                            nc.scalar.dma_start(out=v1[32 * bx:32 * bx + 32, :],
                                                in_=t1v[32 * gb:32 * gb + 32, :, :])
                    np_ = 64 * len(ybs)
                    h1c = sb.tile([128, 7 * 512], bf16, tag="vm")
                    nc.vector.tensor_tensor(out=h1c[0:np_, :], in0=v0[0:np_, :],
                                            in1=v1[0:np_, :], op=OP.max)
                    h1cv = h1c.rearrange("p (x n) -> p x n", x=7)
                    for bx in range(np_ // 32):
                        nc.sync.dma_start(
                            out=h1_dram[:, 2 * ybp + bx, 7 * xh:7 * xh + 7, :],
                            in_=h1cv[32 * bx:32 * bx + 32, :, :])
                    if ybp in (4, 6):
                        emit_gram_ch(2 * ((ybp - 4) // 2) + xh)

            # ============================================================
            # Stage 2: conv2 + maxpool -> h2 [32, 8, 9, 512]; 4 n-chunks
            # ============================================================
            w2a = sb.tile([128, 3, 128], bf16, tag="w2a")
            w2b = sb.tile([128, 3, 128], bf16, tag="w2b")
            nc.sync.dma_start(out=w2a, in_=w2l_ext[0:128, :, :])
            nc.sync.dma_start(out=w2b[0:65, :, :], in_=w2l_ext[128:193, :, :])
            # conv2: loop yb-pairs, 1 yb per patch; full n; patches [*, 16x, 512]
            for ybp in (0, 2):
                t2s = []
                for yb in (ybp, ybp + 1):
                    p2at = sb.tile([128, 16 * 512], bf16, tag="cp", bufs=2,
                                   name=f"p2a{yb}")
                    p2a = p2at
                    p2bt = sb.tile([128, 16 * 512], bf16, tag="cp2b",
                                   name=f"p2b{yb}")
                    p2b = p2bt[0:65, :]
                    p2av = p2a.rearrange("p (x n) -> p x n", x=16)
                    p2bv = p2b.rearrange("p (x n) -> p x n", x=16)
                    nc.sync.dma_start(out=p2b[64:65, :],
                                      in_=odram[0:1, 0:16 * 512])
                    for yoff in range(6):
                        y_in = 4 * yb + yoff - 1
                        dst, r0 = (p2av, yoff * 32) if yoff < 4 \
                            else (p2bv, (yoff - 4) * 32)
                        if 0 <= y_in <= 13:
                            nc.gpsimd.dma_start(
                                out=dst[r0:r0 + 32, 0:1, :],
                                in_=zdram[0:32, 0:512].rearrange(
                                    "p (x n) -> p x n", x=1))
                            nc.gpsimd.dma_start(
                                out=dst[r0:r0 + 32, 15:16, :],
                                in_=zdram[0:32, 0:512].rearrange(
                                    "p (x n) -> p x n", x=1))
                            nc.gpsimd.dma_start(
                                out=dst[r0:r0 + 32, 1:15, :],
                                in_=h1_dram[:, y_in, :, :])
                        else:
                            nc.gpsimd.dma_start(
                                out=dst[r0:r0 + 32, :, :],
                                in_=zdram[0:32, 0:16 * 512].rearrange(
                                    "p (x n) -> p x n", x=16))
                    t2 = sb.tile([128, 7 * 512], bf16, tag="t1", bufs=2,
                                 name=f"t2_{yb}")
                    t2v = t2.rearrange("p (x n) -> p x n", x=7)
                    for (g0, g1) in ((0, 4), (4, 8), (8, 12), (12, 14)):
                        nxp = g1 - g0
                        pg = ps.tile([128, 2048], f32, tag="big", bufs=1)
                        for xs in range(g0, g1):
                            for kx in range(3):
                                nc.tensor.matmul(
                                    pg[:, (xs - g0) * 512:(xs - g0 + 1) * 512],
                                    w2a[:, kx, :], p2av[:, xs + kx, :],
                                    start=(kx == 0), stop=False)
                            for kx in range(3):
                                nc.tensor.matmul(
                                    pg[:, (xs - g0) * 512:(xs - g0 + 1) * 512],
                                    w2b[0:65, kx, :], p2bv[0:65, xs + kx, :],
                                    start=False, stop=(kx == 2))
                        e2 = sb.tile([128, 2048], bf16, tag="e1", bufs=2)
                        nc.scalar.activation(out=e2[:, 0:nxp * 512],
                                             in_=pg[:, 0:nxp * 512],
                                             func=AF.Relu)
                        e2v = e2.rearrange("p (x n) -> p x n", x=4)
                        nc.vector.tensor_tensor(
                            out=t2v[:, g0 // 2:g1 // 2, :],
                            in0=e2v[:, 0:nxp:2, :], in1=e2v[:, 1:nxp:2, :],
                            op=OP.max)
                    t2s.append(t2v)
                v0 = sb.tile([128, 7 * 512], bf16, tag="v0")
                v1 = sb.tile([128, 7 * 512], bf16, tag="v1")
                for j, t2v in enumerate(t2s):
                    for i, (ga, gb) in enumerate(((0, 1), (2, 3))):
                        bx = 2 * j + i
                        nc.sync.dma_start(out=v0[32 * bx:32 * bx + 32, :],
                                          in_=t2v[32 * ga:32 * ga + 32, :, :])
                        nc.gpsimd.dma_start(out=v1[32 * bx:32 * bx + 32, :],
                                            in_=t2v[32 * gb:32 * gb + 32, :, :])
                h2c = sb.tile([128, 7 * 512], bf16, tag="vm")
                nc.vector.tensor_tensor(out=h2c, in0=v0, in1=v1, op=OP.max)
                h2cv = h2c.rearrange("p (x n) -> p x n", x=7)
                for bx in range(4):
                    y_out = 2 * ybp + bx
                    if y_out <= 6:
                        nc.sync.dma_start(out=h2_dram[:, y_out, :, :],
                                          in_=h2cv[32 * bx:32 * bx + 32, :, :])
                emit_gram_ch(4 + ybp)
                emit_gram_ch(5 + ybp)

            # ============================================================
            # Stage 3: conv3 (7x7x32 -> 7x7x16)  M = (yg4, co16) = 64
            # ============================================================
            w3a = sb.tile([128, 3, 64], bf16, tag="w3a")
            w3b = sb.tile([128, 3, 64], bf16, tag="w3b")
            nc.sync.dma_start(out=w3a, in_=w3l_ext[0:128, :, :])
            nc.sync.dma_start(out=w3b[0:65, :, :], in_=w3l_ext[128:193, :, :])
            F3 = 2 * 9 * 512
            p3a = sb.tile([128, F3], bf16, tag="cp", bufs=2)
            p3bt = sb.tile([128, F3], bf16, tag="cp2b")
            p3b = p3bt[0:65, :]
            p3av = p3a.rearrange("p (yb x n) -> p yb x n", yb=2, x=9)
            p3bv = p3b.rearrange("p (yb x n) -> p yb x n", yb=2, x=9)
            nc.sync.dma_start(out=p3b[64:65, :], in_=odram[0:1, 0:F3])
            for yoff in range(6):
                dst, r0 = (p3av, yoff * 32) if yoff < 4 else (p3bv, (yoff - 4) * 32)
                for yb_ in range(2):
                    y_in = 4 * yb_ + yoff - 1
                    if 0 <= y_in <= 6:
                        nc.gpsimd.dma_start(
                            out=dst[r0:r0 + 32, yb_, 0:1, :],
                            in_=zdram[0:32, 0:512].rearrange(
                                "p (x n) -> p x n", x=1))
                        nc.gpsimd.dma_start(
                            out=dst[r0:r0 + 32, yb_, 8:9, :],
                            in_=zdram[0:32, 0:512].rearrange(
                                "p (x n) -> p x n", x=1))
                        nc.gpsimd.dma_start(
                            out=dst[r0:r0 + 32, yb_, 1:8, :],
                            in_=h2_dram[:, y_in, :, :])
                    else:
                        nc.gpsimd.dma_start(
                            out=dst[r0:r0 + 32, yb_, :, :],
                            in_=zdram[0:32, 0:9 * 512].rearrange(
                                "p (x n) -> p x n", x=9))
            h3t = sb.tile([128, 2 * 7 * 512], bf16, tag="h3")
            h3 = h3t[0:64, :]
            h3v = h3.rearrange("p (yb x n) -> p yb x n", yb=2, x=7)
            for yb in range(2):
                for (x0, x1) in ((0, 4), (4, 7)):
                    pg = ps.tile([128, (x1 - x0) * 512], f32, tag="big", bufs=1)
                    for xi in range(x0, x1):
                        for kx in range(3):
                            nc.tensor.matmul(
                                pg[0:64, (xi - x0) * 512:(xi - x0 + 1) * 512],
                                w3a[:, kx, :], p3av[:, yb, xi + kx, :],
                                start=(kx == 0), stop=False)
                        for kx in range(3):
                            nc.tensor.matmul(
                                pg[0:64, (xi - x0) * 512:(xi - x0 + 1) * 512],
                                w3b[0:65, kx, :], p3bv[0:65, yb, xi + kx, :],
                                start=False, stop=(kx == 2))
                    nc.scalar.activation(
                        out=h3v[:, yb, x0:x1, :],
                        in_=pg[0:64, 0:(x1 - x0) * 512], func=AF.Relu)

            # ============================================================
            # Stage 4: dense 784->16, E, se, AllGather E, scales
            # ============================================================
            dks = []
            for yg in range(4):
                for yb in range(2):
                    y = 4 * yb + yg
                    if y > 6:
                        continue
                    dkt = sb.tile([128, 16], bf16, tag=f"dk{yg}{yb}")
                    dk = dkt[0:112, :]
                    nc.sync.dma_start(out=dk, in_=dwl_ext[112 * y:112 * (y + 1), :])
                    dks.append((yg, yb, dk))
            dbiast = sb.tile([128, 16], bf16, tag="dbias")
            dbias = dbiast[0:1, :]
            nc.sync.dma_start(out=dbias, in_=dwl_ext[784:785, :])
            ones1t = sb.tile([128, 512], bf16, tag="ones1")
            ones1 = ones1t[0:1, :]
            nc.vector.memset(ones1, 1.0)

            pe_ps = ps.tile([128, 512], f32, tag="big", bufs=1)
            first = True
            for (yg, yb, dk) in dks:
                rht = sb.tile([128, 512], bf16, tag="t1", bufs=2)
                rh = rht[0:112, :]
                for co in range(16):
                    nc.sync.dma_start(
                        out=rh.rearrange("(x co) n -> x co n", co=16)[:, co, :],
                        in_=h3v[16 * yg + co:16 * yg + co + 1, yb, :, :])
                nc.tensor.matmul(pe_ps[0:16, :], dk, rh, start=first, stop=False)
                first = False
            nc.tensor.matmul(pe_ps[0:16, :], dbias, ones1, start=False, stop=True)

            shardEt = sb.tile([128, 512], f32, tag="shardE")
            shardE = shardEt[0:17, :]
            nc.scalar.activation(out=shardE[0:16, :], in_=pe_ps[0:16, :], func=AF.Copy)
            E2t = sb.tile([128, 512], f32, tag="E2")
            E2 = E2t[0:16, :]
            nc.vector.tensor_tensor(out=E2, in0=shardE[0:16, :], in1=shardE[0:16, :],
                                    op=OP.mult)
            ones16t = sb.tile([128, 1], f32, tag="ones16")
            ones16 = ones16t[0:16, :]
            nc.vector.memset(ones16, 1.0)
            se_ps = ps.tile([128, 512], f32, tag="big", bufs=1)
            nc.tensor.matmul(se_ps[0:1, :], ones16, E2, start=True, stop=True)
            se_sbt = sb.tile([128, 512], f32, tag="se_sb")
            nc.scalar.activation(out=se_sbt[0:1, :], in_=se_ps[0:1, :], func=AF.Copy)
            nc.sync.dma_start(out=shardE[16:17, :], in_=se_sbt[0:1, :])
            nc.sync.dma_start(out=sharde_dram[:, :], in_=shardE)
            if dbg:
                nc.sync.dma_start(out=dbgE_ext[:, :], in_=shardE)
                nc.sync.dma_start(out=dbgH_ext[:, :], in_=h3v[:, 0, :, :])
            nc.gpsimd.collective_compute(
                "AllGather", OP.bypass, replica_groups=RG,
                ins=[sharde_dram[:, :].opt()], outs=[age_dram[:, :, :].opt()])
            Eallt = sb.tile([128, NCORES * 512], f32, tag="Eall")
            Eall = Eallt[0:17, :]
            for r_ in range(NCORES):
                nc.sync.dma_start(
                    out=Eall[:, 512 * r_:512 * (r_ + 1)],
                    in_=age_dram[r_, :, :])

            smt = sb.tile([128, 4], f32, tag="sm")
            sm = smt[0:1, :]
            sev = sb.tile([128, 32], f32, tag="sev")
            for r_ in range(NCORES):
                nc.sync.dma_start(
                    out=sev[:, 4 * r_:4 * r_ + 4],
                    in_=age_dram[r_, 16, :].rearrange("(c p) -> p c", p=128))
            if dbg:
                nc.sync.dma_start(out=dbgS_ext[:, :], in_=sev)
            sev1 = sb.tile([128, 1], f32, tag="sev1")
            nc.vector.reduce_max(sev1, sev, axis=AX.X)
            nc.sync.dma_start(out=sev_dram[:, :], in_=sev1)
            sev1T = sb.tile([128, 128], f32, tag="sev1T")
            nc.sync.dma_start(out=sev1T[0:1, :],
                              in_=sev_dram[:, :].rearrange("p o -> o p"))
            nc.vector.reduce_max(sm[0:1, 0:1], sev1T[0:1, :], axis=AX.X)
            nc.vector.reciprocal(sm[0:1, 1:2], sm[0:1, 0:1])
            nc.vector.tensor_scalar_mul(sm[0:1, 2:3], sm[0:1, 1:2], QMAX / 2.0)
            nc.vector.tensor_scalar_mul(sm[0:1, 3:4], sm[0:1, 0:1], 2.0 / QMAX)
            s_bc = sb.tile([128, 3], f32)
            nc.gpsimd.partition_broadcast(s_bc[:, 0:1], sm[0:1, 2:3])
            nc.gpsimd.partition_broadcast(s_bc[:, 1:2], sm[0:1, 3:4])
            nc.gpsimd.partition_broadcast(s_bc[:, 2:3], sm[0:1, 0:1])
            seq_t = sb.tile([128, NT], f32)
            for t_ in range(NT):
                nc.sync.dma_start(
                    out=seq_t[:, t_:t_ + 1],
                    in_=shardE[16:17, 128 * t_:128 * (t_ + 1)].rearrange(
                        "a (p o) -> a p o", o=1))
            seoff = sb.tile([128, NT], f32)
            nc.vector.tensor_tensor(out=seoff, in0=seq_t,
                                    in1=s_bc[:, 2:3].to_broadcast([128, NT]),
                                    op=OP.subtract)

            # ============================================================
            # Stage 5: enc quant + stuffing + topk + finish
            # ============================================================
            ones128 = sb.tile([128, 1], f32, tag="ones128")
            nc.vector.memset(ones128, 1.0)
            rsums = sb.tile([128, NT], f32)
            lmaxs = sb.tile([128, NT], f32)
            vi_all = sb.tile([128, 64 * NT], f32)
            ve_all = sb.tile([128, 64 * NT], f32)
            for mg in ((0, 1), (2, 3)):
                for m in mg:
                    stuff = sb.tile([128, 4096], u32, tag="stf", bufs=2,
                                    name=f"stuffsb{m}")
                    nc.sync.dma_start(out=stuff, in_=stuff_dram[m][:, :])
                    stuff_f = stuff.bitcast(f32)
                    lhet = sb.tile([128, 128], f32, tag="lhe", bufs=2)
                    lhe = lhet[0:17, :]
                    nc.scalar.activation(out=lhe[0:16, :],
                                         in_=shardE[0:16, 128 * m:128 * (m + 1)],
                                         func=AF.Copy, scale=-2.0)
                    nc.gpsimd.dma_start(out=lhe[16:17, :], in_=ones1[0:1, 0:128])
                    for ch in range(2):
                        gpe = ps.tile([128, 2048], f32, tag="big", bufs=1)
                        for nn in range(4):
                            col = 2048 * ch + 512 * nn
                            nc.tensor.matmul(
                                gpe[:, 512 * nn:512 * (nn + 1)], lhe,
                                Eall[:, col:col + 512], start=True, stop=True)
                        qc = sb.tile([128, 2048], u32, tag="cp2b")
                        nc.scalar.activation(out=qc, in_=gpe, func=AF.Copy,
                                             scale=s_bc[:, 0:1], bias=511.5)
                        nc.vector.tensor_tensor(
                            out=stuff[:, 2048 * ch:2048 * (ch + 1)],
                            in0=stuff[:, 2048 * ch:2048 * (ch + 1)], in1=qc,
                            op=OP.bitwise_or)
                    # top-64
                    cand = sb.tile([128, 256], f32, tag="cand_a")
                    cand_b = sb.tile([128, 256], f32, tag="cand_b")
                    for gidx in range(32):
                        nc.vector.max(cand[:, 8 * gidx:8 * (gidx + 1)],
                                      stuff_f[:, 128 * gidx:128 * (gidx + 1)])
                    vals = sb.tile([128, 64], f32, tag="vals")
                    cur, nxt = cand, cand_b
                    for r8 in range(8):
                        nc.vector.max(vals[:, 8 * r8:8 * (r8 + 1)], cur)
                        if r8 < 7:
                            nc.vector.match_replace(nxt, vals[:, 8 * r8:8 * (r8 + 1)],
                                                    cur, -1.0)
                            cur, nxt = nxt, cur
                    if dbg and m == 0:
                        nc.sync.dma_start(out=dbgV_ext[:, :], in_=vals)
                    # finish: decode pairs
                    bits = vals.bitcast(u32)
                    fin = sb.tile([128, 64], u32, tag="fin")
                    nc.vector.tensor_scalar(out=fin, in0=bits, scalar1=MASK_HI,
                                            scalar2=None, op0=OP.bitwise_and)
                    addc = sb.tile([128, 1], f32, tag="addc")
                    nc.vector.tensor_scalar_add(addc, sq_q[:, m:m + 1],
                                                HALF_BUCKET - C_SHIFT)
                    vi = vi_all[:, 64 * m:64 * (m + 1)]
                    nc.vector.tensor_tensor(out=vi, in0=fin.bitcast(f32),
                                            in1=addc.to_broadcast([128, 64]),
                                            op=OP.add)
                    nc.scalar.activation(out=vi, in_=vi, func=AF.Sqrt)
                    qu = sb.tile([128, 64], u32, tag="qu")
                    nc.vector.tensor_scalar(out=qu, in0=bits, scalar1=QMAX,
                                            scalar2=None, op0=OP.bitwise_and)
                    qf = sb.tile([128, 64], f32, tag="qf")
                    nc.vector.tensor_copy(qf, qu)
                    ve = ve_all[:, 64 * m:64 * (m + 1)]
                    nc.vector.tensor_scalar(out=ve, in0=qf, scalar1=s_bc[:, 1:2],
                                            scalar2=None, op0=OP.mult)
                    nc.vector.tensor_tensor(
                        out=ve, in0=ve,
                        in1=seoff[:, m:m + 1].to_broadcast([128, 64]), op=OP.add)
                    nc.vector.tensor_scalar_max(ve, ve, 1e-12)
                    nc.scalar.activation(out=ve, in_=ve, func=AF.Sqrt)
                    rec = sb.tile([128, 64], f32, tag="rec")
                    nc.vector.reciprocal(rec, ve)
                    rat = sb.tile([128, 64], f32, tag="rat")
                    nc.vector.tensor_tensor(out=rat, in0=vi, in1=rec, op=OP.mult)
                    nc.vector.reduce_sum(rsums[:, m:m + 1], rat[:, 1:63], axis=AX.X)

            # multiple via AllReduce
            rtot = sb.tile([128, 1], f32)
            nc.vector.reduce_sum(rtot, rsums, axis=AX.X)
            rp = ps.tile([128, 512], f32, tag="big", bufs=1)
            nc.tensor.matmul(rp[0:1, 0:1], rtot, ones128, start=True, stop=True)
            rs_sbt = sb.tile([128, 2], f32, tag="rs_sb")
            rs_sb = rs_sbt[0:1, :]
            nc.scalar.activation(out=rs_sb[0:1, 0:1], in_=rp[0:1, 0:1], func=AF.Copy)
            nc.sync.dma_start(out=rs_dram[:, :], in_=rs_sb[0:1, 0:1])
            nc.gpsimd.collective_compute(
                "AllReduce", OP.add, replica_groups=RG,
                ins=[rs_dram[:, :].opt()], outs=[rsum_dram[:, :].opt()])
            mult_sbt = sb.tile([128, 1], f32, tag="mult_sb")
            mult_sb = mult_sbt[0:1, :]
            nc.sync.dma_start(out=mult_sb, in_=rsum_dram[:, :])
            nc.vector.tensor_scalar_mul(mult_sb, mult_sb, 1.0 / (N * 62))
            m_bc = sb.tile([128, 1], f32)
            nc.gpsimd.partition_broadcast(m_bc, mult_sb)

            for m in range(NT):
                vi = vi_all[:, 64 * m + 1:64 * m + 63]
                ve = ve_all[:, 64 * m + 1:64 * m + 63]
                red = sb.tile([128, 62], f32, tag="red")
                nc.vector.tensor_scalar(out=red, in0=ve, scalar1=m_bc,
                                        scalar2=None, op0=OP.mult)
                nc.vector.tensor_tensor(out=red, in0=vi, in1=red, op=OP.subtract)
                nc.vector.tensor_tensor(out=red, in0=red, in1=red, op=OP.mult)
                nc.vector.reduce_max(lmaxs[:, m:m + 1], red, axis=AX.X)
            ltot = sb.tile([128, 1], f32)
            nc.vector.reduce_sum(ltot, lmaxs, axis=AX.X)
            lp = ps.tile([128, 512], f32, tag="big", bufs=1)
            nc.tensor.matmul(lp[0:1, 0:1], ltot, ones128, start=True, stop=True)
            outsbt = sb.tile([128, 8], f32, tag="outsb")
            outsb = outsbt[0:1, :]
            nc.vector.memset(outsb, 0.0)
            nc.scalar.activation(out=outsb[0:1, 0:1], in_=lp[0:1, 0:1], func=AF.Copy)
            nc.vector.tensor_copy(outsb[0:1, 1:2], rs_sb[0:1, 0:1])
            nc.vector.tensor_copy(outsb[0:1, 2:3], sm[0:1, 0:1])
            nc.vector.tensor_copy(outsb[0:1, 3:4], mult_sb)
            nc.sync.dma_start(out=out_ext[:, :], in_=outsb)

    nc.finalize()
    return nc


def _prep_weights(cw1, cb1, cw2, cb2, cw3, cb3, dw, db):
    import ml_dtypes
    bf = ml_dtypes.bfloat16

    w1l = np.zeros((7, 3, 128), np.float32)
    for yoff in range(6):
        for kx in range(3):
            for yg in range(4):
                ky = yoff - yg
                if 0 <= ky <= 2:
                    w1l[yoff, kx, 32 * yg:32 * yg + 32] = cw1[ky, kx, 0, :]
    for yg in range(4):
        w1l[6, 0, 32 * yg:32 * yg + 32] = cb1

    def mk_w(cw, cb, co):
        wl = np.zeros((193, 3, 4 * co), np.float32)
        for kx in range(3):
            for yoff in range(6):
                for yg in range(4):
                    ky = yoff - yg
                    if 0 <= ky <= 2:
                        wl[32 * yoff:32 * yoff + 32, kx, co * yg:co * (yg + 1)] = \
                            cw[ky, kx, :, :]
        for yg in range(4):
            wl[192, 0, co * yg:co * (yg + 1)] = cb
        return wl

    w2l = mk_w(cw2, cb2, 32)
    w3l = mk_w(cw3, cb3, 16)
    dwl = np.concatenate([dw, db[None, :]], axis=0).astype(np.float32)
    dwl = dwl.astype(bf)
    idT = np.eye(128, dtype=np.float32)
    return (w1l.astype(bf), w2l.astype(bf), w3l.astype(bf), dwl, idT.astype(bf))


def kernel(**inputs):
    from concourse.bass_utils import run_bass_kernel_spmd

    x = np.asarray(inputs["x"], np.float32)
    nnfactor = int(np.asarray(inputs["nnfactor"]))
    assert x.shape == (N, D) and nnfactor == 64

    w1l, w2l, w3l, dwl, idT = _prep_weights(
        np.asarray(inputs["cw1"], np.float32), np.asarray(inputs["cb1"], np.float32),
        np.asarray(inputs["cw2"], np.float32), np.asarray(inputs["cb2"], np.float32),
        np.asarray(inputs["cw3"], np.float32), np.asarray(inputs["cb3"], np.float32),
        np.asarray(inputs["dw"], np.float32), np.asarray(inputs["db"], np.float32))

    if "nc" not in _CACHE:
        _CACHE["nc"] = _build()
    nc = _CACHE["nc"]

    in_maps = []
    for c in range(NCORES):
        in_maps.append({
            "xq": np.ascontiguousarray(x[RPC * c:RPC * (c + 1)]),
            "w1l": w1l, "w2l": w2l, "w3l": w3l, "dwl": dwl, "idT": idT,
        })
    res = run_bass_kernel_spmd(nc, in_maps, core_ids=list(range(NCORES)),
                               trace=TRACE)
    if TRACE and res.exec_time_ns is not None:
        print(f"HW exec time: {res.exec_time_ns} ns", flush=True)
    _CACHE["last_res"] = res
    loss = sum(float(r["out"][0, 0]) for r in res.results) / N
    return np.float32(loss)

